# revision 18
# baseline (speedup 1.0000x reference)
"""Causal multi-head attention (B=4, T=2048, D=2048, H=16) on 8 Trainium2
NeuronCores via Bass/Tile, SPMD with zero collectives.

Sharding: core c = (batch c//2, head-half c%2). Each core computes Q/K/V
projections for its batch restricted to its 8 heads (no redundant work),
runs causal attention for those heads, and produces the partial output
projection A_own @ Wo[own rows]. The host sums each core pair's partials.

Numerics: all matmul operands are bf16 (f32 PSUM accumulation), which runs
at full PE rate and halves SBUF/DMA pressure. The bk bias is dropped
entirely (it shifts every score of a query row equally -> softmax
invariant); bv is folded into bo' = bv @ Wo + bo host-side (attention rows
sum to 1, so bv passes through); bq is kept (it varies scores across keys).

Per-core pipeline:
  1. x^T via DMA-transpose XBAR (bf16); K^T = Wk^T x^T, Q^T = Wq^T x^T + bq
     (transposed layouts, dh on partitions), V = x Wv (natural layout) --
     all SBUF-resident (32KB each).
  2. per head, 4 query passes of 512: S^T tiles = kt_j^T qt (exact causal
     widths, zero padded columns), exp on ACT (no max subtraction: scaled
     scores are O(1)), triangular mask multiply only on the 128x128
     diagonal block (DVE), AV + ones-vector denominator accumulated in
     PSUM, normalization = DVE reciprocal + Pool broadcast + DVE multiply
     into the bf16 A^T slab.
  3. partial O rows = A^T^T Wo_own + bo' streamed out per 128-row tile.
"""
import numpy as np
import ml_dtypes

import concourse.bacc as bacc
import concourse.mybir as mybir
from concourse.tile import TileContext
from concourse.bass_utils import run_bass_kernel_spmd

F32 = mybir.dt.float32
BF16 = mybir.dt.bfloat16
EXP = mybir.ActivationFunctionType.Exp
MULT = mybir.AluOpType.mult
ADD = mybir.AluOpType.add
NPBF16 = ml_dtypes.bfloat16

PROD_CFG = dict(B=4, T=2048, D=2048, H=16)


def _derived(cfg):
    B, T, D, H = cfg["B"], cfg["T"], cfg["D"], cfg["H"]
    d = dict(cfg)
    d.update(
        HPC=H // 2,          # heads per core (8)
        KC=D // 128,         # contraction chunks (16)
        TP=T // 128,         # key tiles (16)
        NP=T // 512,         # query passes of 512 (4)
        T2=T // 2,           # x^T half width
        N_CORES=2 * B,
    )
    return d


def build_nc(cfg):
    c = _derived(cfg)
    T, D = c["T"], c["D"]
    HPC, KC, TP, NP, T2 = c["HPC"], c["KC"], c["TP"], c["NP"], c["T2"]
    SCALE = float(128 ** -0.5)

    nc = bacc.Bacc(
        "TRN2", target_bir_lowering=False, debug=False, num_devices=c["N_CORES"]
    )
    x_in = nc.dram_tensor("x", [T, D], BF16, kind="ExternalInput").ap()
    wkq_in = nc.dram_tensor("wkq", [128, HPC, 2, KC, 128], BF16,
                            kind="ExternalInput").ap()
    wv_in = nc.dram_tensor("wv", [128, KC, HPC * 128], BF16, kind="ExternalInput").ap()
    wo_in = nc.dram_tensor("wo", [128, HPC, D], BF16, kind="ExternalInput").ap()
    bq_in = nc.dram_tensor("bq", [128, HPC], F32, kind="ExternalInput").ap()
    bo_in = nc.dram_tensor("bo", [1, D], F32, kind="ExternalInput").ap()
    tri_in = nc.dram_tensor("tri", [128, 128], BF16, kind="ExternalInput").ap()
    ones_in = nc.dram_tensor("ones_c", [128, 1], BF16, kind="ExternalInput").ap()
    o = nc.dram_tensor("o", [T, D], F32, kind="ExternalOutput").ap()

    with TileContext(nc) as tc:
        with tc.tile_pool(name="const", bufs=1) as pconst:
            tri = pconst.tile([128, 128], BF16, tag="tri")
            nc.sync.dma_start(out=tri[:], in_=tri_in[:])
            ones = pconst.tile([128, 1], BF16, tag="ones")
            nc.sync.dma_start(out=ones[:], in_=ones_in[:])
            bq_sb = pconst.tile([128, HPC], F32, tag="bq")
            nc.sync.dma_start(out=bq_sb[:], in_=bq_in[:])
            warm = pconst.tile([1, 2], F32, tag="warm")
            nc.scalar.activation(warm[:, 0:1], bq_sb[0:1, 0:1], EXP)
            kt = pconst.tile([128, HPC, T], BF16, tag="kt")
            qt = pconst.tile([128, HPC, T], BF16, tag="qt")
            vsb = pconst.tile([128, TP, HPC * 128], BF16, tag="vsb")

            # ---------------- phase 1: x^T, K^T, Q^T, V ----------------
            with (
                tc.tile_pool(name="xt", bufs=3) as pxt,
                tc.tile_pool(name="wkq", bufs=3) as pw,
                tc.tile_pool(name="wv", bufs=1) as pwv,
                tc.tile_pool(name="ps_kq", bufs=4, space="PSUM") as pkq,
                tc.tile_pool(name="ps_v", bufs=2, space="PSUM") as pv,
            ):
                wvs = pwv.tile([128, KC, HPC * 128], BF16, tag="wvs")
                for half in range(2):
                    h0 = half * T2
                    # x^T quarter tiles (3 rotating bufs): the first K/Q
                    # psum tile needs only one quarter's 8 XBARs -> short
                    # pipeline fill. XBAR transposes stay on the sync
                    # queue (racy on the Activation HWDGE queue); weight
                    # slabs go on the scalar queue.
                    xtq = []
                    for jt in range(2):
                        xt = pxt.tile([128, KC, 512], BF16, tag="xt")
                        q0 = h0 + jt * 512
                        for k in range(KC):
                            nc.sync.dma_start(
                                out=xt[:, k, :],
                                in_=x_in[q0:q0 + 512, k * 128:(k + 1) * 128],
                                transpose=True,
                            )
                        xtq.append(xt)
                        if half == 0 and jt == 0:
                            # wvs load rides the sync queue behind the
                            # first quarter's XBARs; done long before the
                            # first V matmul.
                            nc.sync.dma_start(out=wvs[:], in_=wv_in[:])
                    # K^T and Q^T projections (dh on partitions);
                    # one DMA carries both m-slabs (wk, wq)
                    for m in range(HPC):
                        wpair = pw.tile([128, 2, KC, 128], BF16, tag="w")
                        nc.scalar.dma_start(out=wpair[:], in_=wkq_in[:, m])
                        for wi, (outt, bias) in enumerate(
                            ((kt, None), (qt, bq_sb))
                        ):
                            ws = wpair[:, wi]
                            for jt in range(2):
                                ps = pkq.tile([128, 512], F32, tag="pskq")
                                for k in range(KC):
                                    nc.tensor.matmul(
                                        ps[:],
                                        ws[:, k, :],
                                        xtq[jt][:, k, :],
                                        start=(k == 0),
                                        stop=(k == KC - 1),
                                    )
                                cols = h0 + jt * 512
                                if bias is None:
                                    nc.vector.tensor_copy(
                                        outt[:, m, cols:cols + 512], ps[:]
                                    )
                                else:
                                    nc.vector.tensor_scalar_add(
                                        outt[:, m, cols:cols + 512], ps[:],
                                        bias[:, m:m + 1],
                                    )
                    # V projection (natural layout), x^T chunks stationary
                    for tt in range(T2 // 128):
                        xv = xtq[tt // 4]
                        tc_ = tt % 4
                        psv = pv.tile([128, HPC * 128], F32, tag="psv")
                        for k in range(KC):
                            # one start=True per PSUM bank: 512-wide chunks
                            for nq in range(HPC * 128 // 512):
                                nc.tensor.matmul(
                                    psv[:, nq * 512:(nq + 1) * 512],
                                    xv[:, k, tc_ * 128:(tc_ + 1) * 128],
                                    wvs[:, k, nq * 512:(nq + 1) * 512],
                                    start=(k == 0),
                                    stop=(k == KC - 1),
                                )
                        nc.scalar.copy(
                            vsb[:, half * (T2 // 128) + tt, :], psv[:]
                        )

            # ---------------- phase 2+3: attention per head ----------------
            with (
                tc.tile_pool(name="at", bufs=1) as pat,
                tc.tile_pool(name="wo", bufs=1) as pwo,
            ):
                at = pat.tile([128, HPC, T], BF16, tag="at")
                wos = pwo.tile([128, HPC, D], BF16, tag="wos")
                nc.sync.dma_start(out=wos[:], in_=wo_in[:])
                with (
                    tc.tile_pool(name="pt", bufs=5) as ppt,
                    tc.tile_pool(name="lin", bufs=2) as plin,
                    tc.tile_pool(name="ps_s", bufs=3, space="PSUM") as ps_s,
                    tc.tile_pool(name="ps_a", bufs=1, space="PSUM") as ps_a,
                    tc.tile_pool(name="ps_l", bufs=1, space="PSUM") as ps_l,
                ):
                  DEPTH = 2  # pairs of score tiles issued ahead of AV
                  for h in range(HPC):
                    for p in range(NP):
                        jmax = 4 * p + 4
                        psa = ps_a.tile([128, 512], F32, tag="psa")
                        psl = ps_l.tile([1, 512], F32, tag="psl")

                        def consume(pair):
                            for j, c0, w, pt, off in pair:
                                nc.tensor.matmul(
                                    psa[:, c0:512],
                                    vsb[:, j, h * 128:(h + 1) * 128],
                                    pt[:, off:off + w],
                                    start=(j == 0),
                                    stop=(j == jmax - 1),
                                )
                            # denominators grouped after the pair's AVs:
                            # both share the `ones` stationary (one load)
                            for j, c0, w, pt, off in pair:
                                nc.tensor.matmul(
                                    psl[:, c0:512],
                                    ones[:],
                                    pt[:, off:off + w],
                                    start=(j == 0),
                                    stop=(j == jmax - 1),
                                )

                        pending = []
                        # j-tiles processed in pairs sharing one [128,1024]
                        # PSUM tile (banks don't overlap) and ONE wide exp.
                        for j0 in range(0, jmax, 2):
                            pss = ps_s.tile([128, 1024], F32, tag="pss")
                            pt = ppt.tile([128, 1024], BF16, tag="pt")
                            pair = []
                            for idx, j in enumerate((j0, j0 + 1)):
                                diag = j >= 4 * p
                                c0 = 128 * j - 512 * p if diag else 0
                                w = 512 - c0
                                off = idx * 512 + c0
                                nc.tensor.matmul(
                                    pss[:, off:off + w],
                                    kt[:, h, j * 128:(j + 1) * 128],
                                    qt[:, h, p * 512 + c0:(p + 1) * 512],
                                    start=True,
                                    stop=True,
                                )
                                pair.append((j, c0, w, pt, off))
                            # one exp spanning both tiles (the gap between
                            # them, if any, is exp'd garbage that is never
                            # read downstream)
                            e0 = pair[0][4]
                            nc.scalar.activation(
                                pt[:, e0:1024], pss[:, e0:1024], EXP,
                                scale=SCALE,
                            )
                            for j, c0, w, ptv, off in pair:
                                if j >= 4 * p:
                                    nc.vector.tensor_mul(
                                        ptv[:, off:off + 128],
                                        ptv[:, off:off + 128], tri[:]
                                    )
                            pending.append(pair)
                            if len(pending) > DEPTH:
                                consume(pending.pop(0))
                        for pair in pending:
                            consume(pair)
                        # fast PSUM evacuation: free psa/psl (bufs=1) with
                        # quick DVE copies; the slow normalize chain reads
                        # the SBUF copies off the critical path.
                        a_raw = plin.tile([128, 512], F32, tag="araw")
                        nc.vector.tensor_copy(a_raw[:], psa[:])
                        l_raw = plin.tile([1, 512], F32, tag="lraw")
                        nc.vector.tensor_copy(l_raw[:], psl[:])
                        linv = plin.tile([1, 512], F32, tag="linv")
                        nc.vector.reciprocal_approx_fast(linv[:], l_raw[:])
                        lb = plin.tile([128, 512], F32, tag="lb")
                        nc.gpsimd.partition_broadcast(lb[:], linv[:], channels=128)
                        nc.vector.tensor_tensor(
                            at[:, h, p * 512:(p + 1) * 512], a_raw[:], lb[:], MULT
                        )

                # ---------------- phase 4: partial output projection ----------------
                with (
                    tc.tile_pool(name="ost", bufs=2) as post,
                    tc.tile_pool(name="bo4", bufs=1) as pbo,
                    tc.tile_pool(name="ps_o", bufs=2, space="PSUM") as ps_o,
                ):
                    bo_sb = pbo.tile([1, D], F32, tag="bo")
                    nc.sync.dma_start(out=bo_sb[:], in_=bo_in[:])
                    bo_bc = pbo.tile([128, D], F32, tag="bo_bc")
                    nc.gpsimd.partition_broadcast(bo_bc[:], bo_sb[:], channels=128)
                    for tt in range(T // 128):
                        for nh in range(2):
                            pso = ps_o.tile([128, 2 * 512], F32, tag="pso")
                            for k in range(HPC):
                                for sc in range(2):
                                    nc.tensor.matmul(
                                        pso[:, sc * 512:(sc + 1) * 512],
                                        at[:, k, tt * 128:(tt + 1) * 128],
                                        wos[:, k, nh * 1024 + sc * 512:
                                            nh * 1024 + (sc + 1) * 512],
                                        start=(k == 0),
                                        stop=(k == HPC - 1),
                                    )
                            ost = post.tile([128, 1024], F32, tag="ost")
                            nc.vector.tensor_tensor(
                                ost[:], pso[:],
                                bo_bc[:, nh * 1024:(nh + 1) * 1024], ADD
                            )
                            nc.scalar.dma_start(
                                out=o[tt * 128:(tt + 1) * 128,
                                      nh * 1024:(nh + 1) * 1024],
                                in_=ost[:],
                            )
    nc.compile()
    return nc


def host_prep(cfg, inputs):
    """Per-core input maps (weights sliced per head-half, bf16, DMA-friendly
    layouts)."""
    c = _derived(cfg)
    B, D, HPC, KC = c["B"], c["D"], c["HPC"], c["KC"]
    f32 = np.float32
    x = np.asarray(inputs["x"], f32)
    Wq = np.asarray(inputs["Wq"], f32)
    Wk = np.asarray(inputs["Wk"], f32)
    Wv = np.asarray(inputs["Wv"], f32)
    Wo = np.asarray(inputs["Wo"], f32)
    bq = np.asarray(inputs["bq"], f32)
    bv = np.asarray(inputs["bv"], f32)
    bo = np.asarray(inputs["bo"], f32)

    bo_eff = (bv @ Wo + bo).astype(f32)[None, :]
    bo_zero = np.zeros_like(bo_eff)
    tri = np.ascontiguousarray(np.triu(np.ones((128, 128), f32)).astype(NPBF16))
    ones_c = np.ones((128, 1), NPBF16)

    HW = HPC * 128  # columns per head-half (1024)
    in_maps = []
    for core in range(c["N_CORES"]):
        b, hh = core // 2, core % 2
        cols = slice(hh * HW, (hh + 1) * HW)
        wq_s = Wq[:, cols].astype(NPBF16).reshape(KC, 128, HPC, 128)
        wk_s = Wk[:, cols].astype(NPBF16).reshape(KC, 128, HPC, 128)
        wkq_s = np.stack(
            [wk_s.transpose(1, 2, 0, 3), wq_s.transpose(1, 2, 0, 3)], axis=2
        )
        wv_s = Wv[:, cols].astype(NPBF16).reshape(KC, 128, HW)
        wo_s = Wo[cols, :].astype(NPBF16).reshape(HPC, 128, D)
        bq_s = bq[cols].reshape(HPC, 128).T
        in_maps.append({
            "x": np.ascontiguousarray(x[b].astype(NPBF16)),
            "wkq": np.ascontiguousarray(wkq_s),
            "wv": np.ascontiguousarray(wv_s.transpose(1, 0, 2)),
            "wo": np.ascontiguousarray(wo_s.transpose(1, 0, 2)),
            "bq": np.ascontiguousarray(bq_s),
            "bo": bo_eff if hh == 0 else bo_zero,
            "tri": tri,
            "ones_c": ones_c,
        })
    return in_maps


def run_cores(cfg, nc, in_maps, trace=False, tmpdir=None):
    c = _derived(cfg)
    n = c["N_CORES"]
    res = run_bass_kernel_spmd(
        nc, in_maps, list(range(n)), trace=trace, tmpdir=tmpdir
    )
    B, T, D = c["B"], c["T"], c["D"]
    out = np.empty((B, T, D), dtype=np.float32)
    for b in range(B):
        out[b] = res.results[2 * b]["o"] + res.results[2 * b + 1]["o"]
    return out, res


_NC_CACHE = {}


def kernel(x, Wq, bq, Wk, bk, Wv, bv, Wo, bo):
    cfg = PROD_CFG
    key = tuple(sorted(cfg.items()))
    if key not in _NC_CACHE:
        _NC_CACHE[key] = build_nc(cfg)
    nc = _NC_CACHE[key]
    inputs = dict(x=x, Wq=Wq, bq=bq, Wk=Wk, bk=bk, Wv=Wv, bv=bv, Wo=Wo, bo=bo)
    in_maps = host_prep(cfg, inputs)
    out, _ = run_cores(cfg, nc, in_maps)
    return out


# revision 19
# speedup vs baseline: 1.1659x; 1.1659x over previous
"""Causal multi-head attention (B=4, T=2048, D=2048, H=16) on 8 Trainium2
NeuronCores via Bass/Tile, SPMD with zero collectives.

Sharding: core c = (batch c//2, head-half c%2). Each core computes Q/K/V
projections for its batch restricted to its 8 heads (no redundant work),
runs causal attention for those heads, and produces the partial output
projection A_own @ Wo[own rows]. The host sums each core pair's partials.

Numerics: all matmul operands are bf16 (f32 PSUM accumulation), which runs
at full PE rate and halves SBUF/DMA pressure. The bk bias is dropped
entirely (it shifts every score of a query row equally -> softmax
invariant); bv is folded into bo' = bv @ Wo + bo host-side (attention rows
sum to 1, so bv passes through); bq is kept (it varies scores across keys).

Per-core pipeline:
  1. x^T via DMA-transpose XBAR (bf16); K^T = Wk^T x^T, Q^T = Wq^T x^T + bq
     (transposed layouts, dh on partitions), V = x Wv (natural layout) --
     all SBUF-resident (32KB each).
  2. per head, 4 query passes of 512: S^T tiles = kt_j^T qt (exact causal
     widths, zero padded columns), exp on ACT (no max subtraction: scaled
     scores are O(1)), triangular mask multiply only on the 128x128
     diagonal block (DVE), AV + ones-vector denominator accumulated in
     PSUM, normalization = DVE reciprocal + Pool broadcast + DVE multiply
     into the bf16 A^T slab.
  3. partial O rows = A^T^T Wo_own + bo' streamed out per 128-row tile.
"""
import numpy as np
import ml_dtypes

import concourse.bacc as bacc
import concourse.mybir as mybir
from concourse.tile import TileContext
from concourse.bass_utils import run_bass_kernel_spmd

F32 = mybir.dt.float32
BF16 = mybir.dt.bfloat16
EXP = mybir.ActivationFunctionType.Exp
MULT = mybir.AluOpType.mult
ADD = mybir.AluOpType.add
NPBF16 = ml_dtypes.bfloat16

PROD_CFG = dict(B=4, T=2048, D=2048, H=16)


def _derived(cfg):
    B, T, D, H = cfg["B"], cfg["T"], cfg["D"], cfg["H"]
    d = dict(cfg)
    d.update(
        HPC=H // 2,          # heads per core (8)
        KC=D // 128,         # contraction chunks (16)
        TP=T // 128,         # key tiles (16)
        NP=T // 512,         # query passes of 512 (4)
        T2=T // 2,           # x^T half width
        N_CORES=2 * B,
    )
    return d


def build_nc(cfg):
    c = _derived(cfg)
    T, D = c["T"], c["D"]
    HPC, KC, TP, NP, T2 = c["HPC"], c["KC"], c["TP"], c["NP"], c["T2"]
    SCALE = float(128 ** -0.5)

    nc = bacc.Bacc(
        "TRN2", target_bir_lowering=False, debug=False, num_devices=c["N_CORES"]
    )
    x_in = nc.dram_tensor("x", [T, D], BF16, kind="ExternalInput").ap()
    wq_in = nc.dram_tensor("wq", [128, HPC, KC, 128], BF16, kind="ExternalInput").ap()
    wk_in = nc.dram_tensor("wk", [128, HPC, KC, 128], BF16, kind="ExternalInput").ap()
    wv_in = nc.dram_tensor("wv", [128, KC, HPC * 128], BF16, kind="ExternalInput").ap()
    wo_in = nc.dram_tensor("wo", [128, HPC, D], BF16, kind="ExternalInput").ap()
    bq_in = nc.dram_tensor("bq", [128, HPC], F32, kind="ExternalInput").ap()
    bo_in = nc.dram_tensor("bo", [1, D], F32, kind="ExternalInput").ap()
    tri_in = nc.dram_tensor("tri", [128, 128], BF16, kind="ExternalInput").ap()
    ones_in = nc.dram_tensor("ones_c", [128, 1], BF16, kind="ExternalInput").ap()
    o = nc.dram_tensor("o", [T, D], F32, kind="ExternalOutput").ap()

    with TileContext(nc) as tc:
        with tc.tile_pool(name="const", bufs=1) as pconst:
            tri = pconst.tile([128, 128], BF16, tag="tri")
            nc.sync.dma_start(out=tri[:], in_=tri_in[:])
            ones = pconst.tile([128, 1], BF16, tag="ones")
            nc.sync.dma_start(out=ones[:], in_=ones_in[:])
            bq_sb = pconst.tile([128, HPC], F32, tag="bq")
            nc.sync.dma_start(out=bq_sb[:], in_=bq_in[:])
            warm = pconst.tile([1, 2], F32, tag="warm")
            nc.scalar.activation(warm[:, 0:1], bq_sb[0:1, 0:1], EXP)
            kt = pconst.tile([128, HPC, T], BF16, tag="kt")
            qt = pconst.tile([128, HPC, T], BF16, tag="qt")
            vsb = pconst.tile([128, TP, HPC * 128], BF16, tag="vsb")

            # ---------------- phase 1: x^T, K^T, Q^T, V ----------------
            with (
                tc.tile_pool(name="xt", bufs=3) as pxt,
                tc.tile_pool(name="wkq", bufs=4) as pw,
                tc.tile_pool(name="wv", bufs=1) as pwv,
                tc.tile_pool(name="ps_kq", bufs=4, space="PSUM") as pkq,
                tc.tile_pool(name="ps_v", bufs=2, space="PSUM") as pv,
            ):
                wvs = pwv.tile([128, KC, HPC * 128], BF16, tag="wvs")
                for half in range(2):
                    h0 = half * T2
                    # x^T quarter tiles (3 rotating bufs): the first K/Q
                    # psum tile needs only one quarter's 8 XBARs -> short
                    # pipeline fill. XBAR transposes stay on the sync
                    # queue (racy on the Activation HWDGE queue); weight
                    # slabs go on the scalar queue.
                    xtq = []
                    for jt in range(2):
                        xt = pxt.tile([128, KC, 512], BF16, tag="xt")
                        q0 = h0 + jt * 512
                        for k in range(KC):
                            nc.sync.dma_start(
                                out=xt[:, k, :],
                                in_=x_in[q0:q0 + 512, k * 128:(k + 1) * 128],
                                transpose=True,
                            )
                        xtq.append(xt)
                        if half == 0 and jt == 0:
                            # wvs load rides the sync queue behind the
                            # first quarter's XBARs; done long before the
                            # first V matmul.
                            nc.sync.dma_start(out=wvs[:], in_=wv_in[:])
                    # K^T and Q^T projections (dh on partitions)
                    for m in range(HPC):
                        for w_in, outt, bias in (
                            (wk_in, kt, None),
                            (wq_in, qt, bq_sb),
                        ):
                            ws = pw.tile([128, KC, 128], BF16, tag="w")
                            nc.scalar.dma_start(out=ws[:], in_=w_in[:, m])
                            for jt in range(2):
                                ps = pkq.tile([128, 512], F32, tag="pskq")
                                for k in range(KC):
                                    nc.tensor.matmul(
                                        ps[:],
                                        ws[:, k, :],
                                        xtq[jt][:, k, :],
                                        start=(k == 0),
                                        stop=(k == KC - 1),
                                    )
                                cols = h0 + jt * 512
                                if bias is None:
                                    nc.vector.tensor_copy(
                                        outt[:, m, cols:cols + 512], ps[:]
                                    )
                                else:
                                    nc.vector.tensor_scalar_add(
                                        outt[:, m, cols:cols + 512], ps[:],
                                        bias[:, m:m + 1],
                                    )
                    # V projection (natural layout), x^T chunks stationary
                    for tt in range(T2 // 128):
                        xv = xtq[tt // 4]
                        tc_ = tt % 4
                        psv = pv.tile([128, HPC * 128], F32, tag="psv")
                        for k in range(KC):
                            # one start=True per PSUM bank: 512-wide chunks
                            for nq in range(HPC * 128 // 512):
                                nc.tensor.matmul(
                                    psv[:, nq * 512:(nq + 1) * 512],
                                    xv[:, k, tc_ * 128:(tc_ + 1) * 128],
                                    wvs[:, k, nq * 512:(nq + 1) * 512],
                                    start=(k == 0),
                                    stop=(k == KC - 1),
                                )
                        nc.scalar.copy(
                            vsb[:, half * (T2 // 128) + tt, :], psv[:]
                        )

            # ---------------- phase 2+3: attention per head ----------------
            with (
                tc.tile_pool(name="at", bufs=1) as pat,
                tc.tile_pool(name="wo", bufs=1) as pwo,
            ):
                at = pat.tile([128, HPC, T], BF16, tag="at")
                wos = pwo.tile([128, HPC, D], BF16, tag="wos")
                nc.sync.dma_start(out=wos[:], in_=wo_in[:])
                with (
                    tc.tile_pool(name="pt", bufs=5) as ppt,
                    tc.tile_pool(name="lin", bufs=2) as plin,
                    tc.tile_pool(name="ps_s", bufs=3, space="PSUM") as ps_s,
                    tc.tile_pool(name="ps_a", bufs=1, space="PSUM") as ps_a,
                    tc.tile_pool(name="ps_l", bufs=1, space="PSUM") as ps_l,
                ):
                  DEPTH = 2  # pairs of score tiles issued ahead of AV
                  for h in range(HPC):
                    for p in range(NP):
                        jmax = 4 * p + 4
                        psa = ps_a.tile([128, 512], F32, tag="psa")
                        psl = ps_l.tile([1, 512], F32, tag="psl")

                        def consume(pair):
                            for j, c0, w, pt, off in pair:
                                nc.tensor.matmul(
                                    psa[:, c0:512],
                                    vsb[:, j, h * 128:(h + 1) * 128],
                                    pt[:, off:off + w],
                                    start=(j == 0),
                                    stop=(j == jmax - 1),
                                )
                            # denominators grouped after the pair's AVs:
                            # both share the `ones` stationary (one load)
                            for j, c0, w, pt, off in pair:
                                nc.tensor.matmul(
                                    psl[:, c0:512],
                                    ones[:],
                                    pt[:, off:off + w],
                                    start=(j == 0),
                                    stop=(j == jmax - 1),
                                )

                        pending = []
                        # j-tiles processed in pairs sharing one [128,1024]
                        # PSUM tile (banks don't overlap) and ONE wide exp.
                        for j0 in range(0, jmax, 2):
                            pss = ps_s.tile([128, 1024], F32, tag="pss")
                            pt = ppt.tile([128, 1024], BF16, tag="pt")
                            pair = []
                            for idx, j in enumerate((j0, j0 + 1)):
                                diag = j >= 4 * p
                                c0 = 128 * j - 512 * p if diag else 0
                                w = 512 - c0
                                off = idx * 512 + c0
                                nc.tensor.matmul(
                                    pss[:, off:off + w],
                                    kt[:, h, j * 128:(j + 1) * 128],
                                    qt[:, h, p * 512 + c0:(p + 1) * 512],
                                    start=True,
                                    stop=True,
                                )
                                pair.append((j, c0, w, pt, off))
                            # one exp spanning both tiles (the gap between
                            # them, if any, is exp'd garbage that is never
                            # read downstream)
                            e0 = pair[0][4]
                            nc.scalar.activation(
                                pt[:, e0:1024], pss[:, e0:1024], EXP,
                                scale=SCALE,
                            )
                            for j, c0, w, ptv, off in pair:
                                if j >= 4 * p:
                                    nc.vector.tensor_mul(
                                        ptv[:, off:off + 128],
                                        ptv[:, off:off + 128], tri[:]
                                    )
                            pending.append(pair)
                            if len(pending) > DEPTH:
                                consume(pending.pop(0))
                        for pair in pending:
                            consume(pair)
                        # fast PSUM evacuation: free psa/psl (bufs=1) with
                        # quick DVE copies; the slow normalize chain reads
                        # the SBUF copies off the critical path.
                        a_raw = plin.tile([128, 512], F32, tag="araw")
                        nc.vector.tensor_copy(a_raw[:], psa[:])
                        l_raw = plin.tile([1, 512], F32, tag="lraw")
                        nc.vector.tensor_copy(l_raw[:], psl[:])
                        linv = plin.tile([1, 512], F32, tag="linv")
                        nc.vector.reciprocal_approx_fast(linv[:], l_raw[:])
                        lb = plin.tile([128, 512], F32, tag="lb")
                        nc.gpsimd.partition_broadcast(lb[:], linv[:], channels=128)
                        nc.vector.tensor_tensor(
                            at[:, h, p * 512:(p + 1) * 512], a_raw[:], lb[:], MULT
                        )

                # ---------------- phase 4: partial output projection ----------------
                with (
                    tc.tile_pool(name="ost", bufs=2) as post,
                    tc.tile_pool(name="bo4", bufs=1) as pbo,
                    tc.tile_pool(name="ps_o", bufs=2, space="PSUM") as ps_o,
                ):
                    bo_sb = pbo.tile([1, D], F32, tag="bo")
                    nc.sync.dma_start(out=bo_sb[:], in_=bo_in[:])
                    bo_bc = pbo.tile([128, D], F32, tag="bo_bc")
                    nc.gpsimd.partition_broadcast(bo_bc[:], bo_sb[:], channels=128)
                    for tt in range(T // 128):
                        for nh in range(2):
                            pso = ps_o.tile([128, 2 * 512], F32, tag="pso")
                            for k in range(HPC):
                                for sc in range(2):
                                    nc.tensor.matmul(
                                        pso[:, sc * 512:(sc + 1) * 512],
                                        at[:, k, tt * 128:(tt + 1) * 128],
                                        wos[:, k, nh * 1024 + sc * 512:
                                            nh * 1024 + (sc + 1) * 512],
                                        start=(k == 0),
                                        stop=(k == HPC - 1),
                                    )
                            ost = post.tile([128, 1024], F32, tag="ost")
                            nc.vector.tensor_tensor(
                                ost[:], pso[:],
                                bo_bc[:, nh * 1024:(nh + 1) * 1024], ADD
                            )
                            nc.scalar.dma_start(
                                out=o[tt * 128:(tt + 1) * 128,
                                      nh * 1024:(nh + 1) * 1024],
                                in_=ost[:],
                            )
    nc.compile()
    return nc


def host_prep(cfg, inputs):
    """Per-core input maps (weights sliced per head-half, bf16, DMA-friendly
    layouts)."""
    c = _derived(cfg)
    B, D, HPC, KC = c["B"], c["D"], c["HPC"], c["KC"]
    f32 = np.float32
    x = np.asarray(inputs["x"], f32)
    Wq = np.asarray(inputs["Wq"], f32)
    Wk = np.asarray(inputs["Wk"], f32)
    Wv = np.asarray(inputs["Wv"], f32)
    Wo = np.asarray(inputs["Wo"], f32)
    bq = np.asarray(inputs["bq"], f32)
    bv = np.asarray(inputs["bv"], f32)
    bo = np.asarray(inputs["bo"], f32)

    bo_eff = (bv @ Wo + bo).astype(f32)[None, :]
    bo_zero = np.zeros_like(bo_eff)
    tri = np.ascontiguousarray(np.triu(np.ones((128, 128), f32)).astype(NPBF16))
    ones_c = np.ones((128, 1), NPBF16)

    HW = HPC * 128  # columns per head-half (1024)
    in_maps = []
    for core in range(c["N_CORES"]):
        b, hh = core // 2, core % 2
        cols = slice(hh * HW, (hh + 1) * HW)
        wq_s = Wq[:, cols].astype(NPBF16).reshape(KC, 128, HPC, 128)
        wk_s = Wk[:, cols].astype(NPBF16).reshape(KC, 128, HPC, 128)
        wv_s = Wv[:, cols].astype(NPBF16).reshape(KC, 128, HW)
        wo_s = Wo[cols, :].astype(NPBF16).reshape(HPC, 128, D)
        bq_s = bq[cols].reshape(HPC, 128).T
        in_maps.append({
            "x": np.ascontiguousarray(x[b].astype(NPBF16)),
            "wq": np.ascontiguousarray(wq_s.transpose(1, 2, 0, 3)),
            "wk": np.ascontiguousarray(wk_s.transpose(1, 2, 0, 3)),
            "wv": np.ascontiguousarray(wv_s.transpose(1, 0, 2)),
            "wo": np.ascontiguousarray(wo_s.transpose(1, 0, 2)),
            "bq": np.ascontiguousarray(bq_s),
            "bo": bo_eff if hh == 0 else bo_zero,
            "tri": tri,
            "ones_c": ones_c,
        })
    return in_maps


def run_cores(cfg, nc, in_maps, trace=False, tmpdir=None):
    c = _derived(cfg)
    n = c["N_CORES"]
    res = run_bass_kernel_spmd(
        nc, in_maps, list(range(n)), trace=trace, tmpdir=tmpdir
    )
    B, T, D = c["B"], c["T"], c["D"]
    out = np.empty((B, T, D), dtype=np.float32)
    for b in range(B):
        out[b] = res.results[2 * b]["o"] + res.results[2 * b + 1]["o"]
    return out, res


_NC_CACHE = {}


def kernel(x, Wq, bq, Wk, bk, Wv, bv, Wo, bo):
    cfg = PROD_CFG
    key = tuple(sorted(cfg.items()))
    if key not in _NC_CACHE:
        _NC_CACHE[key] = build_nc(cfg)
    nc = _NC_CACHE[key]
    inputs = dict(x=x, Wq=Wq, bq=bq, Wk=Wk, bk=bk, Wv=Wv, bv=bv, Wo=Wo, bo=bo)
    in_maps = host_prep(cfg, inputs)
    out, _ = run_cores(cfg, nc, in_maps)
    return out


# revision 20
# speedup vs baseline: 1.1887x; 1.0196x over previous
"""Causal multi-head attention (B=4, T=2048, D=2048, H=16) on 8 Trainium2
NeuronCores via Bass/Tile, SPMD with zero collectives.

Sharding: core c = (batch c//2, head-half c%2). Each core computes Q/K/V
projections for its batch restricted to its 8 heads (no redundant work),
runs causal attention for those heads, and produces the partial output
projection A_own @ Wo[own rows]. The host sums each core pair's partials.

Numerics: all matmul operands are bf16 (f32 PSUM accumulation), which runs
at full PE rate and halves SBUF/DMA pressure. The bk bias is dropped
entirely (it shifts every score of a query row equally -> softmax
invariant); bv is folded into bo' = bv @ Wo + bo host-side (attention rows
sum to 1, so bv passes through); bq is kept (it varies scores across keys).

Per-core pipeline:
  1. x^T via DMA-transpose XBAR (bf16); K^T = Wk^T x^T, Q^T = Wq^T x^T + bq
     (transposed layouts, dh on partitions), V = x Wv (natural layout) --
     all SBUF-resident (32KB each).
  2. per head, 4 query passes of 512: S^T tiles = kt_j^T qt (exact causal
     widths, zero padded columns), exp on ACT (no max subtraction: scaled
     scores are O(1)), triangular mask multiply only on the 128x128
     diagonal block (DVE), AV + ones-vector denominator accumulated in
     PSUM, normalization = DVE reciprocal + Pool broadcast + DVE multiply
     into the bf16 A^T slab.
  3. partial O rows = A^T^T Wo_own + bo' streamed out per 128-row tile.
"""
import numpy as np
import ml_dtypes

import concourse.bacc as bacc
import concourse.mybir as mybir
from concourse.tile import TileContext
from concourse.bass_utils import run_bass_kernel_spmd

F32 = mybir.dt.float32
BF16 = mybir.dt.bfloat16
EXP = mybir.ActivationFunctionType.Exp
MULT = mybir.AluOpType.mult
ADD = mybir.AluOpType.add
NPBF16 = ml_dtypes.bfloat16

PROD_CFG = dict(B=4, T=2048, D=2048, H=16)


def _derived(cfg):
    B, T, D, H = cfg["B"], cfg["T"], cfg["D"], cfg["H"]
    d = dict(cfg)
    d.update(
        HPC=H // 2,          # heads per core (8)
        KC=D // 128,         # contraction chunks (16)
        TP=T // 128,         # key tiles (16)
        NP=T // 512,         # query passes of 512 (4)
        T2=T // 2,           # x^T half width
        N_CORES=2 * B,
    )
    return d


def build_nc(cfg):
    c = _derived(cfg)
    T, D = c["T"], c["D"]
    HPC, KC, TP, NP, T2 = c["HPC"], c["KC"], c["TP"], c["NP"], c["T2"]
    SCALE = float(128 ** -0.5)

    nc = bacc.Bacc(
        "TRN2", target_bir_lowering=False, debug=False, num_devices=c["N_CORES"]
    )
    x_in = nc.dram_tensor("x", [T, D], BF16, kind="ExternalInput").ap()
    wq_in = nc.dram_tensor("wq", [128, HPC, KC, 128], BF16, kind="ExternalInput").ap()
    wk_in = nc.dram_tensor("wk", [128, HPC, KC, 128], BF16, kind="ExternalInput").ap()
    wv_in = nc.dram_tensor("wv", [128, KC, HPC * 128], BF16, kind="ExternalInput").ap()
    wo_in = nc.dram_tensor("wo", [128, HPC, D], BF16, kind="ExternalInput").ap()
    bq_in = nc.dram_tensor("bq", [128, HPC], F32, kind="ExternalInput").ap()
    bo_in = nc.dram_tensor("bo", [1, D], F32, kind="ExternalInput").ap()
    tri_in = nc.dram_tensor("tri", [128, 128], BF16, kind="ExternalInput").ap()
    ones_in = nc.dram_tensor("ones_c", [128, 1], BF16, kind="ExternalInput").ap()
    o = nc.dram_tensor("o", [T, D], F32, kind="ExternalOutput").ap()

    with TileContext(nc) as tc:
        with tc.tile_pool(name="const", bufs=1) as pconst:
            tri = pconst.tile([128, 128], BF16, tag="tri")
            nc.sync.dma_start(out=tri[:], in_=tri_in[:])
            ones = pconst.tile([128, 1], BF16, tag="ones")
            nc.sync.dma_start(out=ones[:], in_=ones_in[:])
            bq_sb = pconst.tile([128, HPC], F32, tag="bq")
            nc.sync.dma_start(out=bq_sb[:], in_=bq_in[:])
            kt = pconst.tile([128, HPC, T], BF16, tag="kt")
            qt = pconst.tile([128, HPC, T], BF16, tag="qt")
            vsb = pconst.tile([128, TP, HPC * 128], BF16, tag="vsb")

            # ---------------- phase 1: x^T, K^T, Q^T, V ----------------
            with (
                tc.tile_pool(name="warm", bufs=2, space="PSUM") as pwarm,
            ):
              for wu in range(40):
                  pw_ = pwarm.tile([128, 512], F32, tag="wu")
                  for r in range(4):
                      nc.tensor.matmul(
                          pw_[:, r * 128:(r + 1) * 128], tri[:], tri[:],
                          start=True, stop=True,
                      )
            with (
                tc.tile_pool(name="xt", bufs=2) as pxt,
                tc.tile_pool(name="wkq", bufs=3) as pw,
                tc.tile_pool(name="wv", bufs=1) as pwv,
                tc.tile_pool(name="ps_kq", bufs=4, space="PSUM") as pkq,
                tc.tile_pool(name="ps_v", bufs=2, space="PSUM") as pv,
            ):
                wvs = pwv.tile([128, KC, HPC * 128], BF16, tag="wvs")
                for half in range(2):
                    h0 = half * T2
                    xt = pxt.tile([128, KC, T2], BF16, tag="xt")
                    for k in range(KC):
                        # XBAR transposes stay on the sync queue (racy on
                        # the Activation HWDGE queue); weight slabs go on
                        # the scalar queue so neither starves the other.
                        nc.sync.dma_start(
                            out=xt[:, k, :],
                            in_=x_in[h0:h0 + T2, k * 128:(k + 1) * 128],
                            transpose=True,
                        )
                    if half == 0:
                        # wvs load rides the sync queue behind half-0's
                        # XBARs; done long before the first V matmul.
                        nc.sync.dma_start(out=wvs[:], in_=wv_in[:])
                    # K^T and Q^T projections (dh on partitions)
                    for m in range(HPC):
                        for w_in, outt, bias in (
                            (wk_in, kt, None),
                            (wq_in, qt, bq_sb),
                        ):
                            ws = pw.tile([128, KC, 128], BF16, tag="w")
                            nc.scalar.dma_start(out=ws[:], in_=w_in[:, m])
                            for jt in range(2):
                                ps = pkq.tile([128, 512], F32, tag="pskq")
                                for k in range(KC):
                                    nc.tensor.matmul(
                                        ps[:],
                                        ws[:, k, :],
                                        xt[:, k, jt * 512:(jt + 1) * 512],
                                        start=(k == 0),
                                        stop=(k == KC - 1),
                                    )
                                cols = h0 + jt * 512
                                if bias is None:
                                    nc.vector.tensor_copy(
                                        outt[:, m, cols:cols + 512], ps[:]
                                    )
                                else:
                                    nc.vector.tensor_scalar_add(
                                        outt[:, m, cols:cols + 512], ps[:],
                                        bias[:, m:m + 1],
                                    )
                    # V projection (natural layout), x^T chunks stationary
                    for tt in range(T2 // 128):
                        psv = pv.tile([128, HPC * 128], F32, tag="psv")
                        for k in range(KC):
                            # one start=True per PSUM bank: 512-wide chunks
                            for nq in range(HPC * 128 // 512):
                                nc.tensor.matmul(
                                    psv[:, nq * 512:(nq + 1) * 512],
                                    xt[:, k, tt * 128:(tt + 1) * 128],
                                    wvs[:, k, nq * 512:(nq + 1) * 512],
                                    start=(k == 0),
                                    stop=(k == KC - 1),
                                )
                        nc.scalar.copy(
                            vsb[:, half * (T2 // 128) + tt, :], psv[:]
                        )

            # ---------------- phase 2+3: attention per head ----------------
            with (
                tc.tile_pool(name="at", bufs=1) as pat,
                tc.tile_pool(name="wo", bufs=1) as pwo,
            ):
                at = pat.tile([128, HPC, T], BF16, tag="at")
                wos = pwo.tile([128, HPC, D], BF16, tag="wos")
                nc.sync.dma_start(out=wos[:], in_=wo_in[:])
                with (
                    tc.tile_pool(name="pt", bufs=5) as ppt,
                    tc.tile_pool(name="lin", bufs=2) as plin,
                    tc.tile_pool(name="ps_s", bufs=3, space="PSUM") as ps_s,
                    tc.tile_pool(name="ps_a", bufs=1, space="PSUM") as ps_a,
                    tc.tile_pool(name="ps_l", bufs=1, space="PSUM") as ps_l,
                ):
                  DEPTH = 2  # pairs of score tiles issued ahead of AV
                  for h in range(HPC):
                    for p in range(NP):
                        jmax = 4 * p + 4
                        psa = ps_a.tile([128, 512], F32, tag="psa")
                        psl = ps_l.tile([1, 512], F32, tag="psl")

                        def consume(pair):
                            for j, c0, w, pt, off in pair:
                                nc.tensor.matmul(
                                    psa[:, c0:512],
                                    vsb[:, j, h * 128:(h + 1) * 128],
                                    pt[:, off:off + w],
                                    start=(j == 0),
                                    stop=(j == jmax - 1),
                                )
                            # denominators grouped after the pair's AVs:
                            # both share the `ones` stationary (one load)
                            for j, c0, w, pt, off in pair:
                                nc.tensor.matmul(
                                    psl[:, c0:512],
                                    ones[:],
                                    pt[:, off:off + w],
                                    start=(j == 0),
                                    stop=(j == jmax - 1),
                                )

                        pending = []
                        # j-tiles processed in pairs sharing one [128,1024]
                        # PSUM tile (banks don't overlap) and ONE wide exp.
                        for j0 in range(0, jmax, 2):
                            pss = ps_s.tile([128, 1024], F32, tag="pss")
                            pt = ppt.tile([128, 1024], BF16, tag="pt")
                            pair = []
                            for idx, j in enumerate((j0, j0 + 1)):
                                diag = j >= 4 * p
                                c0 = 128 * j - 512 * p if diag else 0
                                w = 512 - c0
                                off = idx * 512 + c0
                                nc.tensor.matmul(
                                    pss[:, off:off + w],
                                    kt[:, h, j * 128:(j + 1) * 128],
                                    qt[:, h, p * 512 + c0:(p + 1) * 512],
                                    start=True,
                                    stop=True,
                                )
                                pair.append((j, c0, w, pt, off))
                            # one exp spanning both tiles (the gap between
                            # them, if any, is exp'd garbage that is never
                            # read downstream)
                            e0 = pair[0][4]
                            nc.scalar.activation(
                                pt[:, e0:1024], pss[:, e0:1024], EXP,
                                scale=SCALE,
                            )
                            for j, c0, w, ptv, off in pair:
                                if j >= 4 * p:
                                    nc.vector.tensor_mul(
                                        ptv[:, off:off + 128],
                                        ptv[:, off:off + 128], tri[:]
                                    )
                            pending.append(pair)
                            if len(pending) > DEPTH:
                                consume(pending.pop(0))
                        for pair in pending:
                            consume(pair)
                        # fast PSUM evacuation: free psa/psl (bufs=1) with
                        # quick DVE copies; the slow normalize chain reads
                        # the SBUF copies off the critical path.
                        a_raw = plin.tile([128, 512], F32, tag="araw")
                        nc.vector.tensor_copy(a_raw[:], psa[:])
                        l_raw = plin.tile([1, 512], F32, tag="lraw")
                        nc.vector.tensor_copy(l_raw[:], psl[:])
                        linv = plin.tile([1, 512], F32, tag="linv")
                        nc.vector.reciprocal_approx_fast(linv[:], l_raw[:])
                        lb = plin.tile([128, 512], F32, tag="lb")
                        nc.gpsimd.partition_broadcast(lb[:], linv[:], channels=128)
                        nc.vector.tensor_tensor(
                            at[:, h, p * 512:(p + 1) * 512], a_raw[:], lb[:], MULT
                        )

                # ---------------- phase 4: partial output projection ----------------
                with (
                    tc.tile_pool(name="ost", bufs=2) as post,
                    tc.tile_pool(name="bo4", bufs=1) as pbo,
                    tc.tile_pool(name="ps_o", bufs=2, space="PSUM") as ps_o,
                ):
                    bo_sb = pbo.tile([1, D], F32, tag="bo")
                    nc.sync.dma_start(out=bo_sb[:], in_=bo_in[:])
                    bo_bc = pbo.tile([128, D], F32, tag="bo_bc")
                    nc.gpsimd.partition_broadcast(bo_bc[:], bo_sb[:], channels=128)
                    for tt in range(T // 128):
                        for nh in range(2):
                            pso = ps_o.tile([128, 2 * 512], F32, tag="pso")
                            for k in range(HPC):
                                for sc in range(2):
                                    nc.tensor.matmul(
                                        pso[:, sc * 512:(sc + 1) * 512],
                                        at[:, k, tt * 128:(tt + 1) * 128],
                                        wos[:, k, nh * 1024 + sc * 512:
                                            nh * 1024 + (sc + 1) * 512],
                                        start=(k == 0),
                                        stop=(k == HPC - 1),
                                    )
                            ost = post.tile([128, 1024], F32, tag="ost")
                            nc.vector.tensor_tensor(
                                ost[:], pso[:],
                                bo_bc[:, nh * 1024:(nh + 1) * 1024], ADD
                            )
                            nc.scalar.dma_start(
                                out=o[tt * 128:(tt + 1) * 128,
                                      nh * 1024:(nh + 1) * 1024],
                                in_=ost[:],
                            )
    nc.compile()
    return nc


def host_prep(cfg, inputs):
    """Per-core input maps (weights sliced per head-half, bf16, DMA-friendly
    layouts)."""
    c = _derived(cfg)
    B, D, HPC, KC = c["B"], c["D"], c["HPC"], c["KC"]
    f32 = np.float32
    x = np.asarray(inputs["x"], f32)
    Wq = np.asarray(inputs["Wq"], f32)
    Wk = np.asarray(inputs["Wk"], f32)
    Wv = np.asarray(inputs["Wv"], f32)
    Wo = np.asarray(inputs["Wo"], f32)
    bq = np.asarray(inputs["bq"], f32)
    bv = np.asarray(inputs["bv"], f32)
    bo = np.asarray(inputs["bo"], f32)

    bo_eff = (bv @ Wo + bo).astype(f32)[None, :]
    bo_zero = np.zeros_like(bo_eff)
    tri = np.ascontiguousarray(np.triu(np.ones((128, 128), f32)).astype(NPBF16))
    ones_c = np.ones((128, 1), NPBF16)

    HW = HPC * 128  # columns per head-half (1024)
    in_maps = []
    for core in range(c["N_CORES"]):
        b, hh = core // 2, core % 2
        cols = slice(hh * HW, (hh + 1) * HW)
        wq_s = Wq[:, cols].astype(NPBF16).reshape(KC, 128, HPC, 128)
        wk_s = Wk[:, cols].astype(NPBF16).reshape(KC, 128, HPC, 128)
        wv_s = Wv[:, cols].astype(NPBF16).reshape(KC, 128, HW)
        wo_s = Wo[cols, :].astype(NPBF16).reshape(HPC, 128, D)
        bq_s = bq[cols].reshape(HPC, 128).T
        in_maps.append({
            "x": np.ascontiguousarray(x[b].astype(NPBF16)),
            "wq": np.ascontiguousarray(wq_s.transpose(1, 2, 0, 3)),
            "wk": np.ascontiguousarray(wk_s.transpose(1, 2, 0, 3)),
            "wv": np.ascontiguousarray(wv_s.transpose(1, 0, 2)),
            "wo": np.ascontiguousarray(wo_s.transpose(1, 0, 2)),
            "bq": np.ascontiguousarray(bq_s),
            "bo": bo_eff if hh == 0 else bo_zero,
            "tri": tri,
            "ones_c": ones_c,
        })
    return in_maps


def run_cores(cfg, nc, in_maps, trace=False, tmpdir=None):
    c = _derived(cfg)
    n = c["N_CORES"]
    res = run_bass_kernel_spmd(
        nc, in_maps, list(range(n)), trace=trace, tmpdir=tmpdir
    )
    B, T, D = c["B"], c["T"], c["D"]
    out = np.empty((B, T, D), dtype=np.float32)
    for b in range(B):
        out[b] = res.results[2 * b]["o"] + res.results[2 * b + 1]["o"]
    return out, res


_NC_CACHE = {}


def kernel(x, Wq, bq, Wk, bk, Wv, bv, Wo, bo):
    cfg = PROD_CFG
    key = tuple(sorted(cfg.items()))
    if key not in _NC_CACHE:
        _NC_CACHE[key] = build_nc(cfg)
    nc = _NC_CACHE[key]
    inputs = dict(x=x, Wq=Wq, bq=bq, Wk=Wk, bk=bk, Wv=Wv, bv=bv, Wo=Wo, bo=bo)
    in_maps = host_prep(cfg, inputs)
    out, _ = run_cores(cfg, nc, in_maps)
    return out


# revision 21
# speedup vs baseline: 1.2227x; 1.0286x over previous
"""Causal multi-head attention (B=4, T=2048, D=2048, H=16) on 8 Trainium2
NeuronCores via Bass/Tile, SPMD with zero collectives.

Sharding: core c = (batch c//2, head-half c%2). Each core computes Q/K/V
projections for its batch restricted to its 8 heads (no redundant work),
runs causal attention for those heads, and produces the partial output
projection A_own @ Wo[own rows]. The host sums each core pair's partials.

Numerics: all matmul operands are bf16 (f32 PSUM accumulation), which runs
at full PE rate and halves SBUF/DMA pressure. The bk bias is dropped
entirely (it shifts every score of a query row equally -> softmax
invariant); bv is folded into bo' = bv @ Wo + bo host-side (attention rows
sum to 1, so bv passes through); bq is kept (it varies scores across keys).

Per-core pipeline:
  1. x^T via DMA-transpose XBAR (bf16); K^T = Wk^T x^T, Q^T = Wq^T x^T + bq
     (transposed layouts, dh on partitions), V = x Wv (natural layout) --
     all SBUF-resident (32KB each).
  2. per head, 4 query passes of 512: S^T tiles = kt_j^T qt (exact causal
     widths, zero padded columns), exp on ACT (no max subtraction: scaled
     scores are O(1)), triangular mask multiply only on the 128x128
     diagonal block (DVE), AV + ones-vector denominator accumulated in
     PSUM, normalization = DVE reciprocal + Pool broadcast + DVE multiply
     into the bf16 A^T slab.
  3. partial O rows = A^T^T Wo_own + bo' streamed out per 128-row tile.
"""
import numpy as np
import ml_dtypes

import concourse.bacc as bacc
import concourse.mybir as mybir
from concourse.tile import TileContext
from concourse.bass_utils import run_bass_kernel_spmd

F32 = mybir.dt.float32
BF16 = mybir.dt.bfloat16
EXP = mybir.ActivationFunctionType.Exp
MULT = mybir.AluOpType.mult
ADD = mybir.AluOpType.add
NPBF16 = ml_dtypes.bfloat16

PROD_CFG = dict(B=4, T=2048, D=2048, H=16)


def _derived(cfg):
    B, T, D, H = cfg["B"], cfg["T"], cfg["D"], cfg["H"]
    d = dict(cfg)
    d.update(
        HPC=H // 2,          # heads per core (8)
        KC=D // 128,         # contraction chunks (16)
        TP=T // 128,         # key tiles (16)
        NP=T // 512,         # query passes of 512 (4)
        T2=T // 2,           # x^T half width
        N_CORES=2 * B,
    )
    return d


def build_nc(cfg):
    c = _derived(cfg)
    T, D = c["T"], c["D"]
    HPC, KC, TP, NP, T2 = c["HPC"], c["KC"], c["TP"], c["NP"], c["T2"]
    SCALE = float(128 ** -0.5)

    nc = bacc.Bacc(
        "TRN2", target_bir_lowering=False, debug=False, num_devices=c["N_CORES"]
    )
    x_in = nc.dram_tensor("x", [T, D], BF16, kind="ExternalInput").ap()
    wq_in = nc.dram_tensor("wq", [128, HPC, KC, 128], BF16, kind="ExternalInput").ap()
    wk_in = nc.dram_tensor("wk", [128, HPC, KC, 128], BF16, kind="ExternalInput").ap()
    wv_in = nc.dram_tensor("wv", [128, KC, HPC * 128], BF16, kind="ExternalInput").ap()
    wo_in = nc.dram_tensor("wo", [128, HPC, D], BF16, kind="ExternalInput").ap()
    bq_in = nc.dram_tensor("bq", [128, HPC], F32, kind="ExternalInput").ap()
    bo_in = nc.dram_tensor("bo", [1, D], F32, kind="ExternalInput").ap()
    tri_in = nc.dram_tensor("tri", [128, 128], BF16, kind="ExternalInput").ap()
    ones_in = nc.dram_tensor("ones_c", [128, 1], BF16, kind="ExternalInput").ap()
    o = nc.dram_tensor("o", [T, D], F32, kind="ExternalOutput").ap()

    with TileContext(nc) as tc:
        with tc.tile_pool(name="const", bufs=1) as pconst:
            tri = pconst.tile([128, 128], BF16, tag="tri")
            nc.sync.dma_start(out=tri[:], in_=tri_in[:])
            ones = pconst.tile([128, 1], BF16, tag="ones")
            nc.sync.dma_start(out=ones[:], in_=ones_in[:])
            bq_sb = pconst.tile([128, HPC], F32, tag="bq")
            nc.sync.dma_start(out=bq_sb[:], in_=bq_in[:])
            kt = pconst.tile([128, HPC, T], BF16, tag="kt")
            qt = pconst.tile([128, HPC, T], BF16, tag="qt")
            vsb = pconst.tile([128, TP, HPC * 128], BF16, tag="vsb")

            # ---------------- phase 1: x^T, K^T, Q^T, V ----------------
            with (
                tc.tile_pool(name="xt", bufs=2) as pxt,
                tc.tile_pool(name="wkq", bufs=3) as pw,
                tc.tile_pool(name="wv", bufs=1) as pwv,
                tc.tile_pool(name="ps_kq", bufs=4, space="PSUM") as pkq,
                tc.tile_pool(name="ps_v", bufs=2, space="PSUM") as pv,
            ):
                wvs = pwv.tile([128, KC, HPC * 128], BF16, tag="wvs")
                for half in range(2):
                    h0 = half * T2
                    xt = pxt.tile([128, KC, T2], BF16, tag="xt")
                    for k in range(KC):
                        # XBAR transposes stay on the sync queue (racy on
                        # the Activation HWDGE queue); weight slabs go on
                        # the scalar queue so neither starves the other.
                        nc.sync.dma_start(
                            out=xt[:, k, :],
                            in_=x_in[h0:h0 + T2, k * 128:(k + 1) * 128],
                            transpose=True,
                        )
                    if half == 0:
                        # wvs load rides the sync queue behind half-0's
                        # XBARs; done long before the first V matmul.
                        nc.sync.dma_start(out=wvs[:], in_=wv_in[:])
                    # K^T and Q^T projections (dh on partitions)
                    for m in range(HPC):
                        for w_in, outt, bias in (
                            (wk_in, kt, None),
                            (wq_in, qt, bq_sb),
                        ):
                            ws = pw.tile([128, KC, 128], BF16, tag="w")
                            nc.scalar.dma_start(out=ws[:], in_=w_in[:, m])
                            for jt in range(2):
                                ps = pkq.tile([128, 512], F32, tag="pskq")
                                for k in range(KC):
                                    nc.tensor.matmul(
                                        ps[:],
                                        ws[:, k, :],
                                        xt[:, k, jt * 512:(jt + 1) * 512],
                                        start=(k == 0),
                                        stop=(k == KC - 1),
                                    )
                                cols = h0 + jt * 512
                                if bias is None:
                                    nc.vector.tensor_copy(
                                        outt[:, m, cols:cols + 512], ps[:]
                                    )
                                else:
                                    nc.vector.tensor_scalar_add(
                                        outt[:, m, cols:cols + 512], ps[:],
                                        bias[:, m:m + 1],
                                    )
                    # V projection (natural layout), x^T chunks stationary
                    for tt in range(T2 // 128):
                        psv = pv.tile([128, HPC * 128], F32, tag="psv")
                        for k in range(KC):
                            # one start=True per PSUM bank: 512-wide chunks
                            for nq in range(HPC * 128 // 512):
                                nc.tensor.matmul(
                                    psv[:, nq * 512:(nq + 1) * 512],
                                    xt[:, k, tt * 128:(tt + 1) * 128],
                                    wvs[:, k, nq * 512:(nq + 1) * 512],
                                    start=(k == 0),
                                    stop=(k == KC - 1),
                                )
                        nc.scalar.copy(
                            vsb[:, half * (T2 // 128) + tt, :], psv[:]
                        )

            # ---------------- phase 2+3: attention per head ----------------
            with (
                tc.tile_pool(name="at", bufs=1) as pat,
                tc.tile_pool(name="wo", bufs=1) as pwo,
            ):
                at = pat.tile([128, HPC, T], BF16, tag="at")
                wos = pwo.tile([128, HPC, D], BF16, tag="wos")
                nc.sync.dma_start(out=wos[:], in_=wo_in[:])
                with (
                    tc.tile_pool(name="pt", bufs=5) as ppt,
                    tc.tile_pool(name="lin", bufs=2) as plin,
                    tc.tile_pool(name="ps_s", bufs=3, space="PSUM") as ps_s,
                    tc.tile_pool(name="ps_a", bufs=1, space="PSUM") as ps_a,
                    tc.tile_pool(name="ps_l", bufs=1, space="PSUM") as ps_l,
                ):
                  DEPTH = 2  # pairs of score tiles issued ahead of AV
                  for h in range(HPC):
                    for p in range(NP):
                        jmax = 4 * p + 4
                        psa = ps_a.tile([128, 512], F32, tag="psa")
                        psl = ps_l.tile([1, 512], F32, tag="psl")

                        def consume(pair):
                            for j, c0, w, pt, off in pair:
                                nc.tensor.matmul(
                                    psa[:, c0:512],
                                    vsb[:, j, h * 128:(h + 1) * 128],
                                    pt[:, off:off + w],
                                    start=(j == 0),
                                    stop=(j == jmax - 1),
                                )
                            # denominators grouped after the pair's AVs:
                            # both share the `ones` stationary (one load)
                            for j, c0, w, pt, off in pair:
                                nc.tensor.matmul(
                                    psl[:, c0:512],
                                    ones[:],
                                    pt[:, off:off + w],
                                    start=(j == 0),
                                    stop=(j == jmax - 1),
                                )

                        pending = []
                        # j-tiles processed in pairs sharing one [128,1024]
                        # PSUM tile (banks don't overlap) and ONE wide exp.
                        for j0 in range(0, jmax, 2):
                            pss = ps_s.tile([128, 1024], F32, tag="pss")
                            pt = ppt.tile([128, 1024], BF16, tag="pt")
                            pair = []
                            for idx, j in enumerate((j0, j0 + 1)):
                                diag = j >= 4 * p
                                c0 = 128 * j - 512 * p if diag else 0
                                w = 512 - c0
                                off = idx * 512 + c0
                                nc.tensor.matmul(
                                    pss[:, off:off + w],
                                    kt[:, h, j * 128:(j + 1) * 128],
                                    qt[:, h, p * 512 + c0:(p + 1) * 512],
                                    start=True,
                                    stop=True,
                                )
                                pair.append((j, c0, w, pt, off))
                            # one exp spanning both tiles (the gap between
                            # them, if any, is exp'd garbage that is never
                            # read downstream)
                            e0 = pair[0][4]
                            nc.scalar.activation(
                                pt[:, e0:1024], pss[:, e0:1024], EXP,
                                scale=SCALE,
                            )
                            for j, c0, w, ptv, off in pair:
                                if j >= 4 * p:
                                    nc.vector.tensor_mul(
                                        ptv[:, off:off + 128],
                                        ptv[:, off:off + 128], tri[:]
                                    )
                            pending.append(pair)
                            if len(pending) > DEPTH:
                                consume(pending.pop(0))
                        for pair in pending:
                            consume(pair)
                        # fast PSUM evacuation: free psa/psl (bufs=1) with
                        # quick DVE copies; the slow normalize chain reads
                        # the SBUF copies off the critical path.
                        a_raw = plin.tile([128, 512], F32, tag="araw")
                        nc.vector.tensor_copy(a_raw[:], psa[:])
                        l_raw = plin.tile([1, 512], F32, tag="lraw")
                        nc.vector.tensor_copy(l_raw[:], psl[:])
                        linv = plin.tile([1, 512], F32, tag="linv")
                        nc.vector.reciprocal_approx_fast(linv[:], l_raw[:])
                        lb = plin.tile([128, 512], F32, tag="lb")
                        nc.gpsimd.partition_broadcast(lb[:], linv[:], channels=128)
                        nc.vector.tensor_tensor(
                            at[:, h, p * 512:(p + 1) * 512], a_raw[:], lb[:], MULT
                        )

                # ---------------- phase 4: partial output projection ----------------
                with (
                    tc.tile_pool(name="ost", bufs=2) as post,
                    tc.tile_pool(name="bo4", bufs=1) as pbo,
                    tc.tile_pool(name="ps_o", bufs=2, space="PSUM") as ps_o,
                ):
                    bo_sb = pbo.tile([1, D], F32, tag="bo")
                    nc.sync.dma_start(out=bo_sb[:], in_=bo_in[:])
                    bo_bc = pbo.tile([128, D], F32, tag="bo_bc")
                    nc.gpsimd.partition_broadcast(bo_bc[:], bo_sb[:], channels=128)
                    for tt in range(T // 128):
                        for nh in range(2):
                            pso = ps_o.tile([128, 2 * 512], F32, tag="pso")
                            for k in range(HPC):
                                for sc in range(2):
                                    nc.tensor.matmul(
                                        pso[:, sc * 512:(sc + 1) * 512],
                                        at[:, k, tt * 128:(tt + 1) * 128],
                                        wos[:, k, nh * 1024 + sc * 512:
                                            nh * 1024 + (sc + 1) * 512],
                                        start=(k == 0),
                                        stop=(k == HPC - 1),
                                    )
                            ost = post.tile([128, 1024], F32, tag="ost")
                            nc.vector.tensor_tensor(
                                ost[:], pso[:],
                                bo_bc[:, nh * 1024:(nh + 1) * 1024], ADD
                            )
                            nc.scalar.dma_start(
                                out=o[tt * 128:(tt + 1) * 128,
                                      nh * 1024:(nh + 1) * 1024],
                                in_=ost[:],
                            )
    nc.compile()
    return nc


def host_prep(cfg, inputs):
    """Per-core input maps (weights sliced per head-half, bf16, DMA-friendly
    layouts)."""
    c = _derived(cfg)
    B, D, HPC, KC = c["B"], c["D"], c["HPC"], c["KC"]
    f32 = np.float32
    x = np.asarray(inputs["x"], f32)
    Wq = np.asarray(inputs["Wq"], f32)
    Wk = np.asarray(inputs["Wk"], f32)
    Wv = np.asarray(inputs["Wv"], f32)
    Wo = np.asarray(inputs["Wo"], f32)
    bq = np.asarray(inputs["bq"], f32)
    bv = np.asarray(inputs["bv"], f32)
    bo = np.asarray(inputs["bo"], f32)

    bo_eff = (bv @ Wo + bo).astype(f32)[None, :]
    bo_zero = np.zeros_like(bo_eff)
    tri = np.ascontiguousarray(np.triu(np.ones((128, 128), f32)).astype(NPBF16))
    ones_c = np.ones((128, 1), NPBF16)

    HW = HPC * 128  # columns per head-half (1024)
    in_maps = []
    for core in range(c["N_CORES"]):
        b, hh = core // 2, core % 2
        cols = slice(hh * HW, (hh + 1) * HW)
        wq_s = Wq[:, cols].astype(NPBF16).reshape(KC, 128, HPC, 128)
        wk_s = Wk[:, cols].astype(NPBF16).reshape(KC, 128, HPC, 128)
        wv_s = Wv[:, cols].astype(NPBF16).reshape(KC, 128, HW)
        wo_s = Wo[cols, :].astype(NPBF16).reshape(HPC, 128, D)
        bq_s = bq[cols].reshape(HPC, 128).T
        in_maps.append({
            "x": np.ascontiguousarray(x[b].astype(NPBF16)),
            "wq": np.ascontiguousarray(wq_s.transpose(1, 2, 0, 3)),
            "wk": np.ascontiguousarray(wk_s.transpose(1, 2, 0, 3)),
            "wv": np.ascontiguousarray(wv_s.transpose(1, 0, 2)),
            "wo": np.ascontiguousarray(wo_s.transpose(1, 0, 2)),
            "bq": np.ascontiguousarray(bq_s),
            "bo": bo_eff if hh == 0 else bo_zero,
            "tri": tri,
            "ones_c": ones_c,
        })
    return in_maps


def run_cores(cfg, nc, in_maps, trace=False, tmpdir=None):
    c = _derived(cfg)
    n = c["N_CORES"]
    res = run_bass_kernel_spmd(
        nc, in_maps, list(range(n)), trace=trace, tmpdir=tmpdir
    )
    B, T, D = c["B"], c["T"], c["D"]
    out = np.empty((B, T, D), dtype=np.float32)
    for b in range(B):
        out[b] = res.results[2 * b]["o"] + res.results[2 * b + 1]["o"]
    return out, res


_NC_CACHE = {}


def kernel(x, Wq, bq, Wk, bk, Wv, bv, Wo, bo):
    cfg = PROD_CFG
    key = tuple(sorted(cfg.items()))
    if key not in _NC_CACHE:
        _NC_CACHE[key] = build_nc(cfg)
    nc = _NC_CACHE[key]
    inputs = dict(x=x, Wq=Wq, bq=bq, Wk=Wk, bk=bk, Wv=Wv, bv=bv, Wo=Wo, bo=bo)
    in_maps = host_prep(cfg, inputs)
    out, _ = run_cores(cfg, nc, in_maps)
    return out
